# revision 26
# baseline (speedup 1.0000x reference)
"""Distributed causal self-attention for Trainium2 (8 NeuronCores).

Sharding: data-parallel over batch (4) x tensor-parallel over heads (2 groups
of 8 heads), Megatron-style.  Each core computes, for one batch element and 8
heads: qkv projection, causal flash-style attention, and its partial c_proj
contribution.  The TP all-reduce (a 2-way partial sum) is done on the host
during unsharding.

Per-core kernel layout choices:
  - host supplies x transposed (xT [C, T]) so the QKV matmul directly yields
    Q^T / K^T with head_dim on partitions; V is produced in natural [t, d]
    layout from the same resident xT tiles.  No PE transposes anywhere.
  - attention scores are computed transposed ([k, q] with k on partitions):
    softmax then needs no cross-partition reduction -- exp is pointwise, the
    denominator comes from a constant ones-column appended to V, and the
    normalization uses a gpsimd partition-broadcast of 1/denom (broadcast
    reads physical partition 0, hence the small partition-relocation DMAs;
    those ride the gpsimd software-DGE queue so they never delay bulk loads
    on the sync HWDGE queue).
  - no max-subtraction in softmax: logits are ~N(0,1)-scaled, |logit| < ~40
    so fp32 exp cannot overflow.
  - all matmuls run in bf16 with fp32 PSUM accumulation (fp32r on real HW
    is a 2-pass fp32_mode=HIGH/LOW sequence -- 2x slower than bf16 and with
    2x the LDWEIGHTS cost, unlike what the cost model claims).
  - causality: fully-masked k-chunks are skipped, diagonal chunks compute
    only the live q-range, two composite masks handle the four partial
    chunks in two ops.
  - startup: every input DMA is issued up front (fine-grained leading chunks
    so compute starts at wire speed, coarse trailing chunks to bound the
    DIRECT2D issue-rate cost) and the first QKV t-block accumulates
    ci-interleaved across 6 PSUM banks so the PE consumes weight/x chunks
    the moment they land rather than serializing on chunk order.
  - emission interleaves QKV t-blocks / c_proj blocks into the attention
    stream as paced TensorE filler (half-chain granularity so the per-group
    exp-latency bubbles get bridged) with warm-keeper matmuls bridging the
    final normalize-chain drain.
  - partial c_proj outputs are stored bf16 (host sums in fp32): halves the
    output DMA traffic and the PSUM-drain copy cost.
"""

import sys

import numpy as np

sys.path.insert(0, "/opt/trn_rl_repo")

import concourse.bass as bass
import concourse.mybir as mybir
import concourse.tile as tile
from concourse import bacc
from concourse.bass_utils import run_bass_kernel_spmd

import ml_dtypes

# Problem dims
B, T, C, H, HD = 4, 2048, 1024, 16, 64
NCORES, DP, TP = 8, 4, 2
NH = H // TP          # 8 heads per core
CL = NH * HD          # 512 local channel width
TB = 512              # phase-1 t block (att qb needs exactly tb <= qb)
NTB = T // TB         # 4
QB = 512              # attention q block
NQB = T // QB         # 4
KC = 128              # attention k chunk
NCI = C // 128        # 8 contraction chunks

f32 = mybir.dt.float32
f32r = mybir.dt.float32r
bf16 = mybir.dt.bfloat16
EXP = mybir.ActivationFunctionType.Exp


def _r(ap):
    return ap.bitcast(f32r)


def build_nc():
    nc = bacc.Bacc("TRN2", target_bir_lowering=False, debug=False)

    xT_d = nc.declare_dram_parameter("xT", [C, T], bf16, isOutput=False)
    wqkv_d = nc.declare_dram_parameter("wqkv", [C, 3 * CL], bf16, isOutput=False)
    wp_d = nc.declare_dram_parameter("wp", [CL, C], bf16, isOutput=False)
    bqk_d = nc.declare_dram_parameter("bqk", [128, 8], f32, isOutput=False)
    m1_d = nc.declare_dram_parameter("m1", [128, 896], bf16, isOutput=False)
    m2_d = nc.declare_dram_parameter("m2", [128, 384], bf16, isOutput=False)
    out_d = nc.declare_dram_parameter("out", [T, C], bf16, isOutput=True)

    with tile.TileContext(nc) as tc:
        with (
            tc.tile_pool(name="consts", bufs=1) as consts,
            tc.tile_pool(name="wpool", bufs=1) as wpool,
            tc.tile_pool(name="xtp", bufs=1) as xtp,
            tc.tile_pool(name="qktp", bufs=1) as qktp,
            tc.tile_pool(name="vpool", bufs=1) as vpool,
            tc.tile_pool(name="ytp", bufs=2) as ytp,
            tc.tile_pool(name="ytrp", bufs=3) as ytrp,
            tc.tile_pool(name="expp", bufs=4) as expp,
            tc.tile_pool(name="bcp", bufs=2) as bcp,
            tc.tile_pool(name="dsqp", bufs=2) as dsqp,
            tc.tile_pool(name="rs0p", bufs=2) as rs0p,
            tc.tile_pool(name="scp", bufs=2) as scp,
            tc.tile_pool(name="outp", bufs=3) as outp,
            tc.tile_pool(name="ps_mm", bufs=2, space="PSUM") as ps_mm,
            tc.tile_pool(name="ps_sc", bufs=2, space="PSUM") as ps_sc,
            tc.tile_pool(name="ps_yt", bufs=2, space="PSUM") as ps_yt,
        ):
            # ---- all input DMAs up front.  Leading w/x chunks go as small
            # ---- transfers so the first QKV matmuls start at wire speed;
            # ---- the rest as coarse transfers to bound issue cost. --------
            w_sb = wpool.tile([128, NCI, 3 * CL], bf16, tag="w")
            xt0 = xtp.tile([128, NCI, TB], bf16, tag="xt0", name="xt0")

            def xt_load_slice(xt, tb, c0, c1):
                t0 = tb * TB
                nc.sync.dma_start(
                    xt[:, c0:c1, :],
                    xT_d.rearrange("(c p) n -> p c n", p=128)[:, c0:c1, t0 : t0 + TB],
                )

            # w column layout (host-side): [v(512) | q0 k0 | q1 k1 | q2 k2 | q3 k3]
            # (pair-blocks of 256 cols).  x first, then v-weights, then qk
            # pair-blocks in the order the startup passes consume them.
            def w_col(m):
                return CL + (m % 4) * 256 + (0 if m < 4 else 128)

            def w_load(c0, c1):
                nc.sync.dma_start(
                    w_sb[:, c0:c1, :],
                    wqkv_d.rearrange("(c p) n -> p c n", p=128)[:, c0:c1, :],
                )

            # full-row chunk loads (3KB lines, efficient); fine-grained at
            # the front so the ci-interleaved V pass starts at wire speed
            w_load(0, 1)
            xt_load_slice(xt0, 0, 0, 2)
            w_load(1, 2)
            w_load(2, 5)
            xt_load_slice(xt0, 0, 2, 5)
            w_load(5, 8)
            xt_load_slice(xt0, 0, 5, 8)

            # ---- constants -------------------------------------------------
            bqk_sb = consts.tile([128, 8], f32)
            nc.sync.dma_start(bqk_sb[:], bqk_d[:, :])
            m1_sb = consts.tile([128, 896], bf16)
            nc.sync.dma_start(m1_sb[:], m1_d[:, :])
            m2_sb = consts.tile([128, 384], bf16)
            nc.sync.dma_start(m2_sb[:], m2_d[:, :])

            # remaining x t-blocks + proj weight, one coarse DMA each
            xt_tiles = {0: xt0}
            for tb in range(1, NTB):
                xt = xtp.tile([128, NCI, TB], bf16, tag=f"xt{tb}", name=f"xt{tb}")
                xt_load_slice(xt, tb, 0, 8)
                xt_tiles[tb] = xt
            wp_sb = wpool.tile([128, 4, C], bf16, tag="wp")
            nc.sync.dma_start(
                wp_sb[:, :, :], wp_d.rearrange("(c p) n -> p c n", p=128)[:, :, :]
            )

            # ---- persistent activations (Q^T/K^T bf16, V bf16) ------------
            qt_sb = qktp.tile([128, 4, T], bf16)  # head pair 2m,2m+1 -> [.,m,.]
            kt_sb = qktp.tile([128, 4, T], bf16)
            v_sb = vpool.tile([128, T // 128, NH, HD + 1], bf16)
            nc.vector.memset(v_sb[:, :, :, HD : HD + 1], 1.0)  # denom ones col

            # ---- emission units -------------------------------------------
            def emit_qk_group(tb, m, ci0=0, ci1=NCI, ps_holder={}):
                t0 = tb * TB
                xt = xt_tiles[tb]
                if ci0 == 0:
                    ps_holder[(tb, m)] = ps_mm.tile(
                        [128, TB], f32, tag="mm", name=f"qkps{tb}_{m}"
                    )
                ps = ps_holder[(tb, m)]
                for ci in range(ci0, ci1):
                    nc.tensor.matmul(
                        ps[:],
                        w_sb[:, ci, w_col(m) : w_col(m) + 128],
                        xt[:, ci, :],
                        start=(ci == 0),
                        stop=(ci == NCI - 1),
                    )
                if ci1 == NCI:
                    del ps_holder[(tb, m)]
                    dest = qt_sb if m < 4 else kt_sb
                    nc.vector.tensor_scalar_add(
                        dest[:, m % 4, t0 : t0 + TB], ps[:], bqk_sb[:, m : m + 1]
                    )

            def emit_v_group(tb, s, ci0=0, ci1=NCI, ps_holder={}):
                tt = tb * (TB // 128) + s
                xt = xt_tiles[tb]
                if ci0 == 0:
                    ps_holder[(tb, s)] = ps_mm.tile(
                        [128, CL], f32, tag="mm", name=f"vps{tb}_{s}"
                    )
                ps = ps_holder[(tb, s)]
                for ci in range(ci0, ci1):
                    nc.tensor.matmul(
                        ps[:],
                        xt[:, ci, s * 128 : (s + 1) * 128],
                        w_sb[:, ci, 0:CL],
                        start=(ci == 0),
                        stop=(ci == NCI - 1),
                    )
                if ci1 == NCI:
                    del ps_holder[(tb, s)]
                    nc.vector.tensor_copy(
                        v_sb[:, tt, :, 0:HD],
                        ps[:].rearrange("p (h d) -> p h d", d=HD),
                    )

            yt_tiles = {}

            def emit_proj(qb, s, half, ci0=0, ci1=4, ps_holder={}):
                yt_sb = yt_tiles[qb]
                q0 = qb * QB
                trow = q0 + s * 128
                if ci0 == 0:
                    ps_holder[(qb, s, half)] = ps_mm.tile(
                        [128, 512], f32, tag="mm", name=f"pjps{qb}_{s}_{half}"
                    )
                pp = ps_holder[(qb, s, half)]
                for ci in range(ci0, ci1):
                    nc.tensor.matmul(
                        pp[:],
                        yt_sb[:, ci, s * 128 : (s + 1) * 128],
                        wp_sb[:, ci, half * 512 : (half + 1) * 512],
                        start=(ci == 0),
                        stop=(ci == 3),
                    )
                if ci1 == 4:
                    del ps_holder[(qb, s, half)]
                    ot = outp.tile([128, 512], bf16, tag="ot")
                    nc.vector.tensor_copy(ot[:], pp[:])
                    nc.sync.dma_start(
                        out_d[trow : trow + 128, half * 512 : (half + 1) * 512], ot[:]
                    )

            # filler machinery: paced emission of independent PE work inside
            # the attention stream so TensorE never idles (keeps HAM warm).
            # Each entry is (gate, fn): fn must be emitted before attention
            # q-block `gate` starts (gate 99 = no deadline).
            filler_q = []
            credit = [0.0]

            def pump(rate):
                credit[0] += rate
                while credit[0] >= 1.0 and filler_q:
                    filler_q.pop(0)[1]()
                    credit[0] -= 1.0

            def flush_gated(qb):
                keep = []
                for gate, fn in filler_q:
                    if gate <= qb:
                        fn()
                    else:
                        keep.append((gate, fn))
                filler_q[:] = keep

            deferred_norm = []  # one-head-delayed normalize chains

            def emit_att_head(qb, h, rate):
                q0 = qb * QB
                po = (h % 2) * 64
                tm = h // 2
                nfull = q0 // KC
                nchunks = nfull + 4
                yt_sb = yt_tiles[qb]
                yt_ps = ps_yt.tile([65, QB], f32, tag="ytps")

                groups = [[i, i + 1] for i in range(0, nfull, 2)]
                groups.append((nfull, nfull + 1))      # partial pair 1
                groups.append((nfull + 2, nfull + 3))  # partial pair 2

                for gi, g in enumerate(groups):
                    is_pp = gi >= len(groups) - 2
                    cos = [0 if kc < nfull else (kc - nfull) * 128 for kc in g]
                    ns = [QB - co for co in cos]
                    w_tot = sum(ns)
                    sc = ps_sc.tile([128, w_tot], f32, tag="sc")
                    off = 0
                    for kc, co, n in zip(g, cos, ns):
                        k0 = kc * KC
                        nc.tensor.matmul(
                            sc[:, off : off + n],
                            kt_sb[po : po + 64, tm, k0 : k0 + KC],
                            qt_sb[po : po + 64, tm, q0 + co : q0 + QB],
                            start=True,
                            stop=True,
                        )
                        off += n
                    ex = expp.tile([128, w_tot], bf16, tag="ex")
                    nc.scalar.activation(ex[:], sc[:], EXP)
                    if is_pp:  # composite mask (triangles + ones spans)
                        msk = m1_sb if gi == len(groups) - 2 else m2_sb
                        nc.vector.tensor_mul(ex[:], ex[:], msk[:, 0:w_tot])
                    off = 0
                    for kc, co, n in zip(g, cos, ns):
                        nc.tensor.matmul(
                            yt_ps[0:65, co:QB],
                            v_sb[:, kc, h, :],
                            ex[:, off : off + n],
                            start=(kc == 0),
                            stop=(kc == nchunks - 1),
                            skip_group_check=True,
                        )
                        off += n
                    pump(rate)
                    if gi == 0 and deferred_norm:
                        # emit the PREVIOUS head's normalize back-half now:
                        # the gpsimd broadcast it depends on has had a full
                        # head of slack, so the DVE mul never head-of-line
                        # blocks the vector queue on it.
                        deferred_norm.pop(0)()

                # normalize: yt[d, q] /= denom[q] (row 64 of yt_ps).
                # One copy releases the PSUM bank immediately; recip and the
                # gpsimd broadcast of 1/denom are issued now, but the final
                # multiplies are deferred one head so the broadcast latency
                # stays off the vector engine's in-order queue.
                # partition_broadcast reads PHYSICAL partition 0, so the
                # reciprocal row is DMA'd there first (engines cannot cross
                # partitions).
                ytr = ytrp.tile([65, QB], f32, tag="ytr")
                nc.vector.tensor_copy(ytr[:], yt_ps[0:65, :])
                # reciprocal of the denom row: spread [1,512] across 16
                # partitions so 16 DVE lanes share the work, then gather back
                # to physical partition 0 for the gpsimd broadcast
                dsq = dsqp.tile([16, QB // 16], f32, tag="dsq")
                nc.sync.dma_start(dsq[:], ytr[64:65, :])
                nc.vector.reciprocal(dsq[:], dsq[:])
                rs0 = rs0p.tile([1, QB], f32, tag="rs0")
                nc.sync.dma_start(rs0[0:1, :], dsq[:])
                bc = bcp.tile([128, QB], f32, tag="bc")
                nc.gpsimd.partition_broadcast(bc[:], rs0[0:1, :])

                def norm_tail(ytr=ytr, bc=bc, yt_sb=yt_sb, tm=tm, po=po):
                    if po == 0:
                        nc.vector.tensor_mul(
                            yt_sb[0:64, tm, :], ytr[0:64, :], bc[0:64, :]
                        )
                    else:
                        sc2 = scp.tile([64, QB], bf16, tag="sc2")
                        nc.vector.tensor_mul(sc2[:], ytr[0:64, :], bc[0:64, :])
                        nc.sync.dma_start(yt_sb[64:128, tm, :], sc2[:])

                deferred_norm.append(norm_tail)

            # ---- interleaved startup --------------------------------------
            # pass 1: the 4 V groups of tb0 accumulate ci-interleaved across
            # 4 PSUM banks (2 mm + 2 borrowed sc) so each w_v/x chunk is
            # consumed the moment its DMA lands.  pass 2: QK for head pair 0
            # only (2 borrowed ytps banks) -- attention q-block 0 can then
            # start ~21us in; QK for pairs 1-3 arrives via filler.
            vacc = [
                ps_mm.tile([128, CL], f32, tag="mm", name="vacc0"),
                ps_mm.tile([128, CL], f32, tag="mm", name="vacc1"),
                ps_yt.tile([128, CL], f32, tag="ytps", name="vacc2"),
                ps_yt.tile([128, CL], f32, tag="ytps", name="vacc3"),
            ]
            qkacc = [
                ps_sc.tile([128, TB], f32, tag="sc", name="qkacc0"),
                ps_sc.tile([128, TB], f32, tag="sc", name="qkacc1"),
            ]
            for ci in range(NCI):
                for s in range(4):
                    nc.tensor.matmul(
                        vacc[s][:],
                        xt0[:, ci, s * 128 : (s + 1) * 128],
                        w_sb[:, ci, 0:CL],
                        start=(ci == 0),
                        stop=(ci == NCI - 1),
                        skip_group_check=True,
                    )
                for k, m in enumerate((0, 4)):
                    nc.tensor.matmul(
                        qkacc[k][:],
                        w_sb[:, ci, w_col(m) : w_col(m) + 128],
                        xt0[:, ci, :],
                        start=(ci == 0),
                        stop=(ci == NCI - 1),
                        skip_group_check=True,
                    )
            # bias-adds first: they unblock attention's first QK (qt/kt pair0
            # AND the sc PSUM slots); V copies follow in chunk order so the
            # first attV group can start before the last copy lands.
            for k, m in enumerate((0, 4)):
                dest = qt_sb if m < 4 else kt_sb
                nc.vector.tensor_scalar_add(
                    dest[:, m % 4, 0:TB], qkacc[k][:], bqk_sb[:, m : m + 1]
                )
            for s in range(4):
                nc.vector.tensor_copy(
                    v_sb[:, s, :, 0:HD],
                    vacc[s][:].rearrange("p (h d) -> p h d", d=HD),
                )

            def qkv_units(tb, ms=(0, 4, 1, 5, 2, 6, 3, 7)):
                # half-chain granularity: pump can bridge exp-latency
                # bubbles with ~0.9us of PE work instead of ~1.7us lumps
                u = []
                for m in ms:
                    u.append(lambda tb=tb, m=m: emit_qk_group(tb, m, 0, 4))
                    u.append(lambda tb=tb, m=m: emit_qk_group(tb, m, 4, 8))
                for s in range(TB // 128):
                    u.append(lambda tb=tb, s=s: emit_v_group(tb, s, 0, 4))
                    u.append(lambda tb=tb, s=s: emit_v_group(tb, s, 4, 8))
                return u

            def proj_units(qb):
                u = []
                for s in range(QB // 128):
                    for half in range(2):
                        u.append(lambda qb=qb, s=s, half=half: emit_proj(qb, s, half, 0, 2))
                        u.append(lambda qb=qb, s=s, half=half: emit_proj(qb, s, half, 2, 4))
                return u

            group_counts = {0: 16, 1: 32}  # groups per q-block
            for qb in range(2):
                # filler available during this qb's attention
                if qb == 0:
                    for m in (1, 5, 2, 6, 3, 7):  # finish tb0 qk pairs 1-3
                        filler_q.append((1, lambda m=m: emit_qk_group(0, m, 0, 4)))
                        filler_q.append((1, lambda m=m: emit_qk_group(0, m, 4, 8)))
                    filler_q.extend((1, u) for u in qkv_units(1))
                else:
                    filler_q.extend((2, u) for u in qkv_units(2))
                flush_gated(qb)
                rate = len(filler_q) / group_counts[qb]
                yt_tiles[qb] = ytp.tile([128, 4, QB], bf16, tag="yt", name=f"yt{qb}")
                for h in (1, 0, 3, 2, 5, 4, 7, 6):
                    emit_att_head(qb, h, rate)

            # ---- merged qb2+qb3 window ------------------------------------
            # qb3's exp load (66us) exceeds its own attention PE (49us);
            # interleaving qb2 and qb3 heads pools both blocks' PE surplus so
            # ScalarE exp never starves the PE locally.  qkv tb3 units carry
            # position deadlines (emitted before the qb3 pair that needs
            # them); proj(2) joins the pool once qb2's last normalize is in.
            flush_gated(3)
            for s in range(4):
                filler_q.append((104, lambda s=s: emit_v_group(3, s, 0, 4)))
                filler_q.append((104, lambda s=s: emit_v_group(3, s, 4, 8)))
            for gate, mpair in ((104, (0, 4)), (108, (1, 5)), (112, (2, 6)), (114, (3, 7))):
                for m in mpair:
                    filler_q.append((gate, lambda m=m: emit_qk_group(3, m, 0, 4)))
                    filler_q.append((gate, lambda m=m: emit_qk_group(3, m, 4, 8)))
            filler_q.extend((199, u) for u in proj_units(0))
            filler_q.extend((199, u) for u in proj_units(1))
            yt_tiles[2] = ytp.tile([128, 4, QB], bf16, tag="yt", name="yt2")
            yt_tiles[3] = ytp.tile([128, 4, QB], bf16, tag="yt", name="yt3")
            merged = [
                (2, 1), (2, 0), (2, 3), (2, 2), (3, 1), (3, 0), (2, 5), (2, 4),
                (3, 3), (3, 2), (2, 7), (2, 6), (3, 5), (3, 4), (3, 7), (3, 6),
            ]
            for idx, (qb, h) in enumerate(merged):
                flush_gated(100 + idx)
                if idx == 13:  # (2,6)'s deferred normalize just landed
                    filler_q.extend((199, u) for u in proj_units(2))
                rem = sum(6 if q == 2 else 8 for q, _ in merged[idx:])
                rate = len(filler_q) / rem
                emit_att_head(qb, h, rate)
            for _, u in filler_q:
                u()
            filler_q[:] = []
            while deferred_norm:
                deferred_norm.pop(0)()
            # warm-keepers: trivial matmuls with no attention deps bridge the
            # remaining PE gap while the last normalize chain drains
            wk_dram = nc.dram_tensor("wk_scratch", [128, 512], f32)
            wk_ps = ps_sc.tile([128, 512], f32, tag="sc")
            for i in range(30):
                nc.tensor.matmul(
                    wk_ps[:], m1_sb[:, 0:128], m1_sb[:, 0:512],
                    start=(i == 0), stop=(i == 29), skip_group_check=True,
                )
            wk_sb = outp.tile([128, 512], f32, tag="wk", bufs=1)
            nc.vector.tensor_copy(wk_sb[:], wk_ps[:])
            nc.sync.dma_start(wk_dram[:, :], wk_sb[:])
            for u in proj_units(3):
                u()
    nc.finalize()
    return nc


_NC_CACHE = {}


def _get_nc():
    if "nc" not in _NC_CACHE:
        _NC_CACHE["nc"] = build_nc()
    return _NC_CACHE["nc"]


def make_in_maps(x, W_qkv, b_qkv, W_proj, b_proj):
    x = np.asarray(x, np.float32)
    W_qkv = np.asarray(W_qkv, np.float32)
    b_qkv = np.asarray(b_qkv, np.float32)
    W_proj = np.asarray(W_proj, np.float32)
    b_proj = np.asarray(b_proj, np.float32)

    tri = np.triu(np.ones((128, 128), np.float32))
    ones128 = np.ones((128, 128), np.float32)
    # composite masks for the two partial-chunk pairs (see build_nc)
    m1 = np.concatenate([tri, ones128, ones128, ones128, tri, ones128, ones128], axis=1).astype(ml_dtypes.bfloat16)
    m2 = np.concatenate([tri, ones128, tri], axis=1).astype(ml_dtypes.bfloat16)
    # softmax rows sum to 1, so the V-bias and proj-bias reduce to one
    # constant output row added host-side: bv_local @ Wp_local summed over
    # both TP groups, plus b_proj.
    extra_row = b_proj.astype(np.float64).copy()
    for g in range(TP):
        h0 = g * NH
        vb = b_qkv[2 * C + h0 * HD : 2 * C + h0 * HD + CL].astype(np.float64)
        extra_row += vb @ W_proj[h0 * HD : h0 * HD + CL, :].astype(np.float64)
    extra_row = extra_row.astype(np.float32)

    in_maps = []
    for core in range(NCORES):
        b = core // TP
        g = core % TP
        h0 = g * NH
        qc = slice(h0 * HD, h0 * HD + CL)
        kc_ = slice(C + h0 * HD, C + h0 * HD + CL)
        vc = slice(2 * C + h0 * HD, 2 * C + h0 * HD + CL)
        # column layout [v | q0 k0 | q1 k1 | q2 k2 | q3 k3] (see build_nc)
        wq = W_qkv[:, qc] * 0.125
        wk_ = W_qkv[:, kc_]
        blocks = [W_qkv[:, vc]]
        for p in range(4):
            blocks.append(wq[:, p * 128 : (p + 1) * 128])
            blocks.append(wk_[:, p * 128 : (p + 1) * 128])
        wqkv = np.ascontiguousarray(
            np.concatenate(blocks, axis=1).astype(ml_dtypes.bfloat16)
        )
        bqk = np.ascontiguousarray(
            np.concatenate([b_qkv[qc] * 0.125, b_qkv[kc_]]).reshape(8, 128).T,
            np.float32,
        )
        wp = np.ascontiguousarray(W_proj[h0 * HD : h0 * HD + CL, :].astype(ml_dtypes.bfloat16))
        xT = np.ascontiguousarray(x[b].T.astype(ml_dtypes.bfloat16))
        in_maps.append(
            {
                "xT": xT,
                "wqkv": wqkv,
                "wp": wp,
                "bqk": bqk,
                "m1": m1,
                "m2": m2,
            }
        )
    return in_maps, extra_row


def kernel(x, W_qkv, b_qkv, W_proj, b_proj, _trace=False, **trace_kwargs):
    nc = _get_nc()
    in_maps, extra_row = make_in_maps(x, W_qkv, b_qkv, W_proj, b_proj)
    res = run_bass_kernel_spmd(
        nc, in_maps, core_ids=list(range(NCORES)), trace=_trace, **trace_kwargs
    )
    outs = [r["out"] for r in res.results]
    y = np.empty((B, T, C), np.float32)
    for b in range(B):
        y[b] = (
            outs[b * TP].astype(np.float32)
            + outs[b * TP + 1].astype(np.float32)
            + extra_row
        )
    if _trace:
        return y, res
    return y
